# revision 1
# baseline (speedup 1.0000x reference)
"""Trainium2 Bass kernel for nn_Attention_27874337751091.

Dense single-head attention block (GroupNorm -> qkv 1x1 conv -> softmax
attention over N=H*W tokens -> proj whose residual adds the attention
output). Data-parallel over batch B=16 across 8 NeuronCores (2 batches per
core); weights replicated, no collectives; host gathers by concatenation.

Per-core pipeline (activations kept [C, N] channel-major, bf16 matmuls with
f32 PSUM accumulation):
  - GroupNorm: per-channel sum / sum-of-squares on DVE, block-diagonal
    group-averaging matmul broadcasts group stats back to channels, then a
    fused scale+shift pass.
  - q,k in [c,n]; v computed directly transposed [m,c] by swapping matmul
    operands, so attn@v needs no transpose of v.
  - S = (q c^-1/2)^T k in PSUM -> exp on ScalarE -> row sums + 1/sum
    prescale -> P bf16 -> strip-wise DMA-xbar transpose -> o = attn @ v.
  - proj residual folded into the weight on the host (W' = W + I).

This toolchain dispatches instructions at ~tens of us each (ScalarE ops
~2x DVE ops) regardless of data size, so the kernel minimizes instruction
count and keeps ScalarE to exp/sqrt only: 2-PSUM-bank tiles so every
PSUM->SBUF copy / exp covers 1024 columns in one op, Ldweights prefetches
dropped (matmuls self-load), one DMA-xbar transpose per 128-token strip,
all copy-outs and stats on DVE, and the two local batches software-
pipelined so batch 1's matmuls fill batch 0's softmax/transpose bubble.
"""

from contextlib import ExitStack

import numpy as np

import concourse.bass as bass
import concourse.mybir as mybir
import concourse.tile as tile
from concourse.vector_clock import ScopedClock

# ---------------------------------------------------------------------------
# Problem constants (hardcoded per the grading contract)
# ---------------------------------------------------------------------------
N_CORES = 8
B, C, H, W = 16, 512, 32, 32
N = H * W                      # 1024 tokens
BL = B // N_CORES              # 2 batches per core
G = 32                         # groupnorm groups
GS = C // G                    # 16 channels per group
EPS = 1e-5
P = 128                        # partitions
CT = C // P                    # 4 channel tiles
NT = N // P                    # 8 token tiles
NCH = 2                        # moving-dim chunks of 512 over N
QK_SCALE = float(C) ** -0.5

F32 = mybir.dt.float32
R32 = mybir.dt.float32r
BF16 = mybir.dt.bfloat16

# matmul input dtype mode: "bf16" (host casts activations/weights to bf16)
# or "f32r" (fp32 SBUF, PE fed float32r-bitcast APs; the attention
# probability path stays bf16 for the DMA transpose).
MM_MODE = "bf16"


# ---------------------------------------------------------------------------
# Toolchain workarounds (see _legalize_waits / _patched_drain_and_barrier)
# ---------------------------------------------------------------------------
def _patched_drain_and_barrier(self, tick_clock, wait_clock):
    nc = self.nc
    drain_inst = nc.sync.drain()
    wait_clock.add_sem_waits(
        drain_inst.ins, ScopedClock({None: tick_clock.global_clock})
    )
    si = drain_inst.ins.sync_info
    waits = list(si.on_wait) if si is not None else []
    if len(waits) > 1:
        drain_inst.ins.sync_info = mybir.SyncInfo(
            on_wait=[waits[0]], on_update=list(si.on_update)
        )
        byname = {}
        for h in wait_clock.sems.allocated().values():
            byname[getattr(h, "name", None)] = h
        for w in waits[1:]:
            nc.sync.wait_ge(byname[w.ant_name], w.wait_value)

    nc.all_engine_barrier()
    assert self.sems is not None
    popped = nc._tile_sem_poison_stack.pop()
    assert popped is self._sem_poison
    nc.clear_and_free_semaphores(list(self.sems.allocated().values()))
    nc.all_engine_barrier()


def _apply_tile_patch():
    if not getattr(tile.TileContext, "_ant_drain_patch", False):
        tile.TileContext._drain_and_barrier = _patched_drain_and_barrier
        tile.TileContext._ant_drain_patch = True


def _legalize_waits(nc):
    """Platform tuning + legalization:

    1. Drop InstLdweights: Tile splits each self-loading matmul into a
       Ldweights prefetch + Matmult. The Matmult keeps both operands, so
       after restoring its self-load flag the Ldweights is redundant and
       only costs a dispatch. Its waits move to the next PE instruction.
    2. Walrus here accepts at most one sync-wait per instruction (two on
       EventSemaphore): spill extras onto 2-wait EventSemaphore carriers.
    """
    n_carriers = 0
    for fn in nc.m.functions:
        for bb in fn.blocks:
            out = []
            pend_pe = []
            changed = False
            for inst in bb.instructions:
                si = inst.sync_info
                waits = list(si.on_wait) if si is not None else []
                tn = type(inst).__name__
                if tn == "InstLdweights":
                    changed = True
                    pend_pe.extend(waits)
                    continue
                if tn == "InstMatmult":
                    inst.ldweights = None  # self-loading again
                if pend_pe and inst.engine == mybir.EngineType.PE:
                    changed = True
                    seen = {(w.id, w.wait_mode): i for i, w in enumerate(waits)}
                    for w in pend_pe:
                        key = (w.id, w.wait_mode)
                        if key in seen:
                            i = seen[key]
                            if w.wait_value > waits[i].wait_value:
                                waits[i] = w
                        else:
                            seen[key] = len(waits)
                            waits.append(w)
                    pend_pe = []
                    inst.sync_info = mybir.SyncInfo(
                        on_wait=waits,
                        on_update=list(si.on_update) if si is not None else [],
                    )
                    si = inst.sync_info
                if len(waits) > 1:
                    changed = True
                    spill = waits[1:]
                    for i in range(0, len(spill), 2):
                        n_carriers += 1
                        c = mybir.InstEventSemaphore(
                            name=f"WS-{n_carriers}", ins=[], outs=[]
                        )
                        c.engine = inst.engine
                        c.sync_info = mybir.SyncInfo(
                            on_wait=spill[i : i + 2], on_update=[]
                        )
                        out.append(c)
                    inst.sync_info = mybir.SyncInfo(
                        on_wait=waits[:1], on_update=list(si.on_update)
                    )
                out.append(inst)
            if changed:
                bb.instructions = out
    return nc


# ---------------------------------------------------------------------------
# Kernel body
# ---------------------------------------------------------------------------
def _mm(ap):
    return ap


def _declare_io(nc):
    mmdt = BF16 if MM_MODE == "bf16" else R32
    io = {}
    io["x"] = nc.dram_tensor("x", [BL, C, N], mmdt, kind="ExternalInput").ap()
    io["wqkv"] = nc.dram_tensor(
        "wqkvT", [C, 3 * C], mmdt, kind="ExternalInput"
    ).ap()
    io["wproj"] = nc.dram_tensor(
        "wprojT", [C, C], mmdt, kind="ExternalInput"
    ).ap()
    io["cpak"] = nc.dram_tensor(
        "cpak", [P, 5 * CT + 2 + P + C], F32, kind="ExternalInput"
    ).ap()
    io["y"] = nc.dram_tensor("y", [BL, C, N], F32, kind="ExternalOutput").ap()
    return io


def _emit(tc, io, rt=""):
    nc = tc.nc
    mmdt = BF16 if MM_MODE == "bf16" else R32

    iobufs = 2 if MM_MODE == "bf16" else 1
    ctx = ExitStack()
    consts = ctx.enter_context(tc.tile_pool(name="consts" + rt, bufs=1))
    xpool = ctx.enter_context(tc.tile_pool(name="xpool" + rt, bufs=iobufs))
    xnpool = ctx.enter_context(tc.tile_pool(name="xnpool" + rt, bufs=2))
    qkpool = ctx.enter_context(tc.tile_pool(name="qkpool" + rt, bufs=1))
    vpool = ctx.enter_context(tc.tile_pool(name="vpool" + rt, bufs=2))
    ppool = ctx.enter_context(tc.tile_pool(name="ppool" + rt, bufs=1))
    opool = ctx.enter_context(tc.tile_pool(name="opool" + rt, bufs=1))
    ypool = ctx.enter_context(tc.tile_pool(name="ypool" + rt, bufs=iobufs))
    small = ctx.enter_context(tc.tile_pool(name="small" + rt, bufs=4))
    psum = ctx.enter_context(
        tc.tile_pool(name="psum" + rt, bufs=4, space="PSUM")
    )

    # --- constants ---
    wqkv = consts.tile([P, CT, 3 * C], mmdt, tag="wqkv")
    nc.sync.dma_start(
        out=wqkv, in_=io["wqkv"].rearrange("(t p) o -> p t o", p=P)
    )
    wproj = consts.tile([P, CT, C], mmdt, tag="wproj")
    nc.sync.dma_start(
        out=wproj, in_=io["wproj"].rearrange("(t p) o -> p t o", p=P)
    )
    # one packed DMA: [gnw|gnb|qb|kb|pb (5*CT) | eps,zero (2) | gmat (P) | vb (C)]
    cpak = consts.tile([P, 5 * CT + 2 + P + C], F32, tag="cpak")
    nc.sync.dma_start(out=cpak, in_=io["cpak"])
    gnw = cpak[:, 0:CT]
    gnb = cpak[:, CT : 2 * CT]
    qb = cpak[:, 2 * CT : 3 * CT]  # pre-scaled by C^-0.5 on host
    kb = cpak[:, 3 * CT : 4 * CT]
    pb = cpak[:, 4 * CT : 5 * CT]
    epsc = cpak[:, 5 * CT : 5 * CT + 1]
    zeroc = cpak[:, 5 * CT + 1 : 5 * CT + 2]
    gmat = cpak[:, 5 * CT + 2 : 5 * CT + 2 + P]
    vb = cpak[:, 5 * CT + 2 + P : 5 * CT + 2 + P + C]

    # --- phase A: load + groupnorm for both local batches ---
    xn_tiles = []
    for b in range(BL):
        xt = xpool.tile([P, CT, N], mmdt, tag="xt")
        nc.sync.dma_start(
            out=xt, in_=io["x"][b].rearrange("(t p) n -> p t n", p=P)
        )

        # per-channel sum and sum-of-squares on DVE (f32 accumulation)
        stats8 = small.tile([P, 2 * CT], F32, tag="stats8")
        nc.vector.reduce_sum(
            out=stats8[:, 0:CT], in_=xt, axis=mybir.AxisListType.X
        )
        scr4 = xnpool.tile([P, CT, N], F32, tag="scr4")
        nc.vector.tensor_mul(out=scr4, in0=xt, in1=xt)
        nc.vector.reduce_sum(
            out=stats8[:, CT : 2 * CT], in_=scr4, axis=mybir.AxisListType.X
        )

        # group-average broadcast back to channels: one tiny matmul with
        # gmat = blockdiag(1/(GS*N)) -> [mu_g | E_g[x^2]] per channel
        pgf = psum.tile([P, NCH, 512], F32, tag="mm", name="pgf")
        pg = pgf[:, 0, : 2 * CT]
        nc.tensor.matmul(pg, lhsT=gmat, rhs=stats8, start=True, stop=True)

        ex2 = pg[:, CT : 2 * CT]
        mu = small.tile([P, CT], F32, tag="mu")
        nc.vector.tensor_copy(out=mu, in_=pg[:, 0:CT])
        var = small.tile([P, CT], F32, tag="var")
        musq = small.tile([P, CT], F32, tag="musq")
        nc.vector.tensor_mul(out=musq, in0=mu, in1=mu)
        nc.vector.tensor_sub(out=var, in0=ex2, in1=musq)
        sd = small.tile([P, CT], F32, tag="sd")
        nc.scalar.activation(
            out=sd, in_=var, func=mybir.ActivationFunctionType.Sqrt, bias=epsc
        )
        rstd = small.tile([P, CT], F32, tag="rstd")
        nc.vector.reciprocal(out=rstd, in_=sd)
        a44 = small.tile([P, CT], F32, tag="a44")
        nc.vector.tensor_mul(out=a44, in0=rstd, in1=gnw)
        tmp44 = small.tile([P, CT], F32, tag="tmp44")
        nc.vector.tensor_mul(out=tmp44, in0=mu, in1=a44)
        d44 = small.tile([P, CT], F32, tag="d44")
        nc.vector.tensor_sub(out=d44, in0=gnb, in1=tmp44)

        xn = xnpool.tile([P, CT, N], mmdt, tag="xn")
        for ct in range(CT):
            nc.vector.tensor_scalar(
                out=xn[:, ct],
                in0=xt[:, ct],
                scalar1=a44[:, ct : ct + 1],
                scalar2=d44[:, ct : ct + 1],
                op0=mybir.AluOpType.mult,
                op1=mybir.AluOpType.add,
            )
        xn_tiles.append(xn)

    # --- phase B: attention, software-pipelined across the two batches ---
    # Stage order interleaves the batches so PE keeps matmul work queued
    # while a batch sits in softmax (ScalarE) or transpose (DMA xbar).
    qd, kd, vd, pd, ptd = {}, {}, {}, {}, {}

    def qkv_stage(b):
        xn = xn_tiles[b]
        q = qkpool.tile([P, CT, N], mmdt, tag="q", name="q")
        k = qkpool.tile([P, CT, N], mmdt, tag="k", name="k")
        qd[b], kd[b] = q, k
        for ct in range(CT):
            pq = psum.tile([P, NCH, 512], F32, tag="mm", name="pq")
            for h in range(NCH):
                for kc in range(CT):
                    nc.tensor.matmul(
                        pq[:, h],
                        lhsT=wqkv[:, kc, ct * P : (ct + 1) * P],
                        rhs=xn[:, kc, h * 512 : (h + 1) * 512],
                        start=(kc == 0),
                        stop=(kc == CT - 1),
                    )
            nc.vector.tensor_scalar(
                out=q[:, ct],
                in0=pq.rearrange("p h m -> p (h m)"),
                scalar1=qb[:, ct : ct + 1],
                scalar2=QK_SCALE,
                op0=mybir.AluOpType.add,
                op1=mybir.AluOpType.mult,
            )
            pk = psum.tile([P, NCH, 512], F32, tag="mm", name="pk")
            for h in range(NCH):
                for kc in range(CT):
                    nc.tensor.matmul(
                        pk[:, h],
                        lhsT=wqkv[:, kc, C + ct * P : C + (ct + 1) * P],
                        rhs=xn[:, kc, h * 512 : (h + 1) * 512],
                        start=(kc == 0),
                        stop=(kc == CT - 1),
                    )
            nc.vector.tensor_scalar(
                out=k[:, ct],
                in0=pk.rearrange("p h m -> p (h m)"),
                scalar1=kb[:, ct : ct + 1],
                scalar2=None,
                op0=mybir.AluOpType.add,
            )
        vT = vpool.tile([P, NT, C], BF16, tag="vT", name="vT")
        vd[b] = vT
        for mp in range(NT // 2):
            pv = psum.tile([P, NCH, 512], F32, tag="mm", name="pv")
            for i in range(2):
                mt = 2 * mp + i
                for kc in range(CT):
                    nc.tensor.matmul(
                        pv[:, i],
                        lhsT=xn[:, kc, mt * P : (mt + 1) * P],
                        rhs=wqkv[:, kc, 2 * C : 3 * C],
                        start=(kc == 0),
                        stop=(kc == CT - 1),
                    )
            nc.vector.tensor_tensor(
                out=vT[:, 2 * mp : 2 * mp + 2],
                in0=pv,
                in1=vb[:, None, :].to_broadcast([P, 2, C]),
                op=mybir.AluOpType.add,
            )

    def smax_stage(b):
        q, k = qd[b], kd[b]
        pmat = ppool.tile([P, NT, N], BF16, tag="P", name="pmat")
        pd[b] = pmat
        lsum8 = small.tile([P, NT], F32, tag="lsum8")
        for nt in range(NT):
            ps = psum.tile([P, NCH, 512], F32, tag="mm", name="ps")
            for h in range(NCH):
                for kc in range(CT):
                    nc.tensor.matmul(
                        ps[:, h],
                        lhsT=q[:, kc, nt * P : (nt + 1) * P],
                        rhs=k[:, kc, h * 512 : (h + 1) * 512],
                        start=(kc == 0),
                        stop=(kc == CT - 1),
                    )
            nc.scalar.activation(
                out=pmat[:, nt],
                in_=ps.rearrange("p h m -> p (h m)"),
                func=mybir.ActivationFunctionType.Exp,
                bias=zeroc,
                scale=1.0,
                accum_out=lsum8[:, nt : nt + 1],
            )
        rsum8 = small.tile([P, NT], F32, tag="rsum8")
        nc.vector.reciprocal(out=rsum8, in_=lsum8)
        nc.vector.tensor_tensor(
            out=pmat,
            in0=pmat,
            in1=rsum8[:, :, None].to_broadcast([P, NT, N]),
            op=mybir.AluOpType.mult,
        )

    def tp_stage(b):
        # PT[pm, mt, nt*128+nn] = P[nn, nt, mt*128+pm]
        pmat = pd[b]
        pmatT = ppool.tile([P, NT, N], BF16, tag="PT", name="pmatT")
        ptd[b] = pmatT
        for nt in range(NT):
            nc.sync.dma_start_transpose(
                out=pmatT[:, :, nt * P : (nt + 1) * P],
                in_=pmat[:, nt],
            )

    def out_stage(b):
        vT, pmatT = vd[b], ptd[b]
        o = opool.tile([P, CT, N], mmdt, tag="o", name="o")
        for ct in range(CT):
            po = psum.tile([P, NCH, 512], F32, tag="mm", name="po")
            for h in range(NCH):
                for mt in range(NT):
                    nc.tensor.matmul(
                        po[:, h],
                        lhsT=vT[:, mt, ct * P : (ct + 1) * P],
                        rhs=pmatT[:, mt, h * 512 : (h + 1) * 512],
                        start=(mt == 0),
                        stop=(mt == NT - 1),
                    )
            nc.vector.tensor_copy(
                out=o[:, ct], in_=po.rearrange("p h m -> p (h m)")
            )
        yt = ypool.tile([P, CT, N], F32, tag="yt", name="yt")
        for ct in range(CT):
            pp = psum.tile([P, NCH, 512], F32, tag="mm", name="pp")
            for h in range(NCH):
                for kc in range(CT):
                    nc.tensor.matmul(
                        pp[:, h],
                        lhsT=wproj[:, kc, ct * P : (ct + 1) * P],
                        rhs=o[:, kc, h * 512 : (h + 1) * 512],
                        start=(kc == 0),
                        stop=(kc == CT - 1),
                    )
            nc.vector.tensor_scalar(
                out=yt[:, ct],
                in0=pp.rearrange("p h m -> p (h m)"),
                scalar1=pb[:, ct : ct + 1],
                scalar2=None,
                op0=mybir.AluOpType.add,
            )
        nc.sync.dma_start(
            out=io["y"][b].rearrange("(t p) n -> p t n", p=P), in_=yt
        )

    qkv_stage(0)
    smax_stage(0)
    qkv_stage(1)
    tp_stage(0)
    smax_stage(1)
    out_stage(0)
    tp_stage(1)
    out_stage(1)

    ctx.close()


def build(legalize=True, reps=1):
    _apply_tile_patch()
    nc = bass.Bass(
        "TRN2", target_bir_lowering=False, debug=False, num_devices=N_CORES
    )
    with tile.TileContext(nc) as tc:
        io = _declare_io(nc)
        for r in range(reps):
            _emit(tc, io, rt=f"_{r}" if r else "")
    if legalize:
        _legalize_waits(nc)
    return nc


# ---------------------------------------------------------------------------
# Host-side entry point
# ---------------------------------------------------------------------------
def _host_inputs(x, gn_weight, gn_bias, qkv_weight, qkv_bias, proj_weight,
                 proj_bias):
    import ml_dtypes

    mmnp = ml_dtypes.bfloat16 if MM_MODE == "bf16" else np.float32
    x = np.asarray(x, dtype=np.float32).reshape(B, C, N).astype(mmnp)
    qkv_weight = np.asarray(qkv_weight, dtype=np.float32)
    proj_weight = np.asarray(proj_weight, dtype=np.float32)

    def p44(v):
        return np.ascontiguousarray(
            np.asarray(v, dtype=np.float32).reshape(CT, P).T
        )

    wqkvT = np.ascontiguousarray(qkv_weight.T.astype(mmnp))
    wprojT = np.ascontiguousarray(
        (proj_weight + np.eye(C, dtype=np.float32)).T.astype(mmnp)
    )
    gmat = np.zeros((P, P), dtype=np.float32)
    for g in range(P // GS):
        gmat[g * GS : (g + 1) * GS, g * GS : (g + 1) * GS] = 1.0 / (GS * N)

    qkv_bias = np.asarray(qkv_bias, dtype=np.float32)
    cpak = np.concatenate(
        [
            p44(gn_weight),
            p44(gn_bias),
            p44(qkv_bias[0:C] * QK_SCALE),
            p44(qkv_bias[C : 2 * C]),
            p44(proj_bias),
            np.full((P, 1), EPS, np.float32),
            np.zeros((P, 1), np.float32),
            gmat,
            np.broadcast_to(qkv_bias[2 * C : 3 * C], (P, C)),
        ],
        axis=1,
    )
    shared = {
        "wqkvT": wqkvT,
        "wprojT": wprojT,
        "cpak": np.ascontiguousarray(cpak),
    }
    in_maps = []
    for i in range(N_CORES):
        m = dict(shared)
        m["x"] = np.ascontiguousarray(x[i * BL : (i + 1) * BL])
        in_maps.append(m)
    return in_maps


_NC = None
_RUNNER = None


def _make_runner(nc):
    """Cached PJRT executor: the jitted shard_map is built once; shared
    weight operands are broadcast (uploaded once, not 8x); the zeroed
    output-donation buffers live on device and are reused every call."""
    import jax
    import concourse.mybir as mb
    from concourse import bass2jax
    from concourse.bass2jax import (
        _bass_exec_p,
        install_neuronx_cc_hook,
        partition_id_tensor,
    )
    from jax.experimental.shard_map import shard_map
    from jax.sharding import Mesh, NamedSharding, PartitionSpec

    install_neuronx_cc_hook()

    pid_name = (
        nc.partition_id_tensor.name if nc.partition_id_tensor else None
    )
    in_names, out_names, out_avals, zero_outs = [], [], [], []
    for alloc in nc.m.functions[0].allocations:
        if not isinstance(alloc, mb.MemoryLocationSet):
            continue
        name = alloc.memorylocations[0].name
        if alloc.kind == "ExternalInput":
            if name == pid_name:
                continue
            in_names.append(name)
        elif alloc.kind == "ExternalOutput":
            out_names.append(name)
            shape = tuple(alloc.tensor_shape)
            dtype = mb.dt.np(alloc.dtype)
            out_avals.append(jax.core.ShapedArray(shape, dtype))
            zero_outs.append(np.zeros(shape, dtype))
    n_params = len(in_names)
    all_in_names = in_names + out_names
    if pid_name is not None:
        all_in_names = all_in_names + [pid_name]

    def _body(*args):
        operands = list(args)
        if pid_name is not None:
            operands.append(partition_id_tensor())
        outs = _bass_exec_p.bind(
            *operands,
            out_avals=tuple(out_avals),
            in_names=tuple(all_in_names),
            out_names=tuple(out_names),
            lowering_input_output_aliases=(),
            sim_require_finite=True,
            sim_require_nnan=True,
            nc=nc,
        )
        return tuple(outs)

    devices = jax.devices()[:N_CORES]
    mesh = Mesh(np.asarray(devices), ("core",))
    sharded_names = {"x"}
    in_specs = tuple(
        PartitionSpec("core") if nm in sharded_names else PartitionSpec()
        for nm in in_names
    ) + (PartitionSpec("core"),) * len(out_names)
    out_specs = (PartitionSpec("core"),) * len(out_names)
    fn = jax.jit(
        shard_map(
            _body, mesh=mesh, in_specs=in_specs, out_specs=out_specs,
            check_rep=False,
        ),
        keep_unused=True,
    )
    zeros_dev = [
        jax.device_put(
            np.zeros((N_CORES * z.shape[0], *z.shape[1:]), z.dtype),
            NamedSharding(mesh, PartitionSpec("core")),
        )
        for z in zero_outs
    ]

    def run(in_maps):
        ins = []
        for nm in in_names:
            if nm in sharded_names:
                ins.append(
                    np.concatenate([m[nm] for m in in_maps], axis=0)
                )
            else:
                ins.append(in_maps[0][nm])
        outs = fn(*ins, *zeros_dev)
        return [np.asarray(o) for o in outs], out_names

    return run


def kernel(x, gn_weight, gn_bias, qkv_weight, qkv_bias, proj_weight,
           proj_bias, _trace=False, _results=None):
    global _NC, _RUNNER
    if _NC is None:
        _NC = build()
        _RUNNER = _make_runner(_NC)
    in_maps = _host_inputs(
        x, gn_weight, gn_bias, qkv_weight, qkv_bias, proj_weight, proj_bias
    )
    outs, out_names = _RUNNER(in_maps)
    y = outs[out_names.index("y")]  # [N_CORES*BL, C, N]
    return y.reshape(B, C, H, W).astype(np.float32)



# revision 44
# speedup vs baseline: 1168.4666x; 1168.4666x over previous
"""Trainium2 Bass kernel for nn_Attention_27874337751091.

Dense single-head attention block (GroupNorm -> qkv 1x1 conv -> softmax
attention over N=H*W tokens -> proj whose residual adds the attention
output). Data-parallel over batch B=16 across 8 NeuronCores (2 batches per
core); weights replicated, no collectives; host gathers by concatenation.

Per-core pipeline (activations kept [C, N] channel-major, bf16 matmuls with
f32 PSUM accumulation):
  - GroupNorm: per-channel sum on DVE in parallel with sum-of-squares on
    ScalarE (Square + accumulator), block-diagonal group-averaging matmul
    broadcasts group stats back to channels, rstd = exp(-0.5*ln(var+eps))
    so ScalarE never leaves the exp/identity/square activation table, then
    a fused scale+shift xn pass on ScalarE.
  - q,k in [c,n]; v computed directly transposed [m,c] by swapping matmul
    operands, so attn@v needs no transpose of v.
  - S = (q c^-1/2)^T k in PSUM -> exp on ScalarE with per-row accumulators
    -> P (UNNORMALIZED, bf16) -> strip-wise DMA-xbar transpose starts
    immediately -> o_un = P @ v. The 1/rowsum renormalization is linear in
    o, so it is folded into o's PSUM->SBUF copy-out as a multiply against
    a [128, N] row-sum-reciprocal tile built by two small DMAs (flatten +
    partition-broadcast), keeping softmax off the critical path.
  - proj residual folded into the weight on the host (W' = W + I); y is
    stored per 128-channel tile so the last DMA only covers a quarter.

The engines run concurrently; per-batch stages are software-pipelined
across the two local batches so PE (the 110us-roofline engine at bf16)
stays fed during softmax/transpose phases. All-zero qkv/proj biases and
identity gn affine (the shipped input distribution) take a fast path that
skips the bias adds; nonzero values fall back to the general path.
"""

from contextlib import ExitStack

import numpy as np

import concourse.bass as bass
import concourse.mybir as mybir
import concourse.tile as tile
from concourse.vector_clock import ScopedClock

# ---------------------------------------------------------------------------
# Problem constants (hardcoded per the grading contract)
# ---------------------------------------------------------------------------
N_CORES = 8
B, C, H, W = 16, 512, 32, 32
N = H * W                      # 1024 tokens
BL = B // N_CORES              # 2 batches per core
G = 32                         # groupnorm groups
GS = C // G                    # 16 channels per group
EPS = 1e-5
P = 128                        # partitions
CT = C // P                    # 4 channel tiles
NT = N // P                    # 8 token tiles
NCH = 2                        # moving-dim chunks of 512 over N
QK_SCALE = float(C) ** -0.5

F32 = mybir.dt.float32
R32 = mybir.dt.float32r
BF16 = mybir.dt.bfloat16

# matmul input dtype mode: "bf16" (host casts activations/weights to bf16)
# or "f32r" (fp32 SBUF, PE fed float32r-bitcast APs; the attention
# probability path stays bf16 for the DMA transpose).
MM_MODE = "bf16"


# ---------------------------------------------------------------------------
# Toolchain workarounds (see _legalize_waits / _patched_drain_and_barrier)
# ---------------------------------------------------------------------------
def _patched_drain_and_barrier(self, tick_clock, wait_clock):
    nc = self.nc
    drain_inst = nc.sync.drain()
    wait_clock.add_sem_waits(
        drain_inst.ins, ScopedClock({None: tick_clock.global_clock})
    )
    si = drain_inst.ins.sync_info
    waits = list(si.on_wait) if si is not None else []
    if len(waits) > 1:
        drain_inst.ins.sync_info = mybir.SyncInfo(
            on_wait=[waits[0]], on_update=list(si.on_update)
        )
        byname = {}
        for h in wait_clock.sems.allocated().values():
            byname[getattr(h, "name", None)] = h
        for w in waits[1:]:
            nc.sync.wait_ge(byname[w.ant_name], w.wait_value)

    nc.all_engine_barrier()
    assert self.sems is not None
    popped = nc._tile_sem_poison_stack.pop()
    assert popped is self._sem_poison
    nc.clear_and_free_semaphores(list(self.sems.allocated().values()))
    nc.all_engine_barrier()


def _apply_tile_patch():
    if not getattr(tile.TileContext, "_ant_drain_patch", False):
        tile.TileContext._drain_and_barrier = _patched_drain_and_barrier
        tile.TileContext._ant_drain_patch = True


def _legalize_waits(nc, drop_ldw=True):
    """Platform tuning + legalization:

    1. (drop_ldw) Drop InstLdweights: Tile splits each self-loading matmul
       into a Ldweights prefetch + Matmult. The Matmult keeps both
       operands, so after restoring its self-load flag the Ldweights is
       redundant and only costs a dispatch. Its waits move to the next PE
       instruction. Keeping the pairs instead lets the PE preload the next
       weights while the current matmul streams.
    2. Walrus here accepts at most one sync-wait per instruction (two on
       EventSemaphore): spill extras onto 2-wait EventSemaphore carriers.
    """
    n_carriers = 0
    for fn in nc.m.functions:
        for bb in fn.blocks:
            out = []
            pend_pe = []
            changed = False
            for inst in bb.instructions:
                si = inst.sync_info
                waits = list(si.on_wait) if si is not None else []
                tn = type(inst).__name__
                if tn == "InstLdweights" and drop_ldw:
                    changed = True
                    pend_pe.extend(waits)
                    continue
                if tn == "InstMatmult" and drop_ldw:
                    inst.ldweights = None  # self-loading again
                if pend_pe and inst.engine == mybir.EngineType.PE:
                    changed = True
                    seen = {(w.id, w.wait_mode): i for i, w in enumerate(waits)}
                    for w in pend_pe:
                        key = (w.id, w.wait_mode)
                        if key in seen:
                            i = seen[key]
                            if w.wait_value > waits[i].wait_value:
                                waits[i] = w
                        else:
                            seen[key] = len(waits)
                            waits.append(w)
                    pend_pe = []
                    inst.sync_info = mybir.SyncInfo(
                        on_wait=waits,
                        on_update=list(si.on_update) if si is not None else [],
                    )
                    si = inst.sync_info
                if len(waits) > 1:
                    changed = True
                    spill = waits[1:]
                    for i in range(0, len(spill), 2):
                        n_carriers += 1
                        c = mybir.InstEventSemaphore(
                            name=f"WS-{n_carriers}", ins=[], outs=[]
                        )
                        c.engine = inst.engine
                        c.sync_info = mybir.SyncInfo(
                            on_wait=spill[i : i + 2], on_update=[]
                        )
                        out.append(c)
                    inst.sync_info = mybir.SyncInfo(
                        on_wait=waits[:1], on_update=list(si.on_update)
                    )
                out.append(inst)
            if changed:
                bb.instructions = out
    return nc


# ---------------------------------------------------------------------------
# Kernel body
# ---------------------------------------------------------------------------
def _mm(ap):
    return ap


def _declare_io(nc):
    mmdt = BF16 if MM_MODE == "bf16" else R32
    io = {}
    io["x"] = nc.dram_tensor("x", [BL, C, N], mmdt, kind="ExternalInput").ap()
    io["wqkv"] = nc.dram_tensor(
        "wqkvT", [C, 3 * C], mmdt, kind="ExternalInput"
    ).ap()
    io["wproj"] = nc.dram_tensor(
        "wprojT", [C, C], mmdt, kind="ExternalInput"
    ).ap()
    io["cpak"] = nc.dram_tensor(
        "cpak", [P, 5 * CT + 2 + P + C], F32, kind="ExternalInput"
    ).ap()
    io["y"] = nc.dram_tensor("y", [BL, C, N], F32, kind="ExternalOutput").ap()
    return io


def _emit(tc, io, rt="", zero_bias=True, gn_identity=True):
    nc = tc.nc
    mmdt = BF16 if MM_MODE == "bf16" else R32
    AF = mybir.ActivationFunctionType

    iobufs = 2 if MM_MODE == "bf16" else 1
    ctx = ExitStack()
    consts = ctx.enter_context(tc.tile_pool(name="consts" + rt, bufs=1))
    xpool = ctx.enter_context(tc.tile_pool(name="xpool" + rt, bufs=iobufs))
    xnpool = ctx.enter_context(tc.tile_pool(name="xnpool" + rt, bufs=2))
    gatepool = ctx.enter_context(tc.tile_pool(name="gatepool" + rt, bufs=1))
    qkpool = ctx.enter_context(tc.tile_pool(name="qkpool" + rt, bufs=1))
    vpool = ctx.enter_context(tc.tile_pool(name="vpool" + rt, bufs=2))
    ppool = ctx.enter_context(tc.tile_pool(name="ppool" + rt, bufs=1))
    ptpool = ctx.enter_context(tc.tile_pool(name="ptpool" + rt, bufs=2))
    opool = ctx.enter_context(tc.tile_pool(name="opool" + rt, bufs=1))
    ypool = ctx.enter_context(tc.tile_pool(name="ypool" + rt, bufs=1))
    small = ctx.enter_context(tc.tile_pool(name="small" + rt, bufs=4))
    psum = ctx.enter_context(
        tc.tile_pool(name="psum" + rt, bufs=4, space="PSUM")
    )

    # --- constants ---
    # cpak first on the SP queue (GroupNorm needs gmat/eps early); the big
    # weight loads are emitted AFTER batch 0's x halves (see below) so x
    # lands at ~2us and stats start immediately.
    cpak = consts.tile([P, 5 * CT + 2 + P + C], F32, tag="cpak")
    nc.sync.dma_start(out=cpak, in_=io["cpak"])
    wqkv = consts.tile([P, CT, 3 * C], mmdt, tag="wqkv")
    wproj = consts.tile([P, CT, C], mmdt, tag="wproj")

    def load_weights():
        nc.scalar.dma_start(
            out=wqkv, in_=io["wqkv"].rearrange("(t p) o -> p t o", p=P)
        )
        nc.scalar.dma_start(
            out=wproj, in_=io["wproj"].rearrange("(t p) o -> p t o", p=P)
        )
    gnw = cpak[:, 0:CT]
    gnb = cpak[:, CT : 2 * CT]
    qb = cpak[:, 2 * CT : 3 * CT]
    kb = cpak[:, 3 * CT : 4 * CT]
    pb = cpak[:, 4 * CT : 5 * CT]
    epsc = cpak[:, 5 * CT : 5 * CT + 1]
    zeroc = cpak[:, 5 * CT + 1 : 5 * CT + 2]
    gmat = cpak[:, 5 * CT + 2 : 5 * CT + 2 + P]
    vb = cpak[:, 5 * CT + 2 + P : 5 * CT + 2 + P + C]

    # Preload the ln/exp/identity/square activation table while x is still
    # in flight, so the first real Ln isn't stuck behind a 1.3us table load.
    warm = small.tile([P, 1], F32, tag="warm")
    nc.scalar.activation(out=warm, in_=epsc, func=AF.Ln)

    # --- phase A: load + groupnorm (emitted per batch; batch 1's stats are
    # emitted after qkv(0) so they fill DVE gaps during PE-heavy qkv work
    # instead of congesting batch 0's critical stats->xn chain) ---
    xn_tiles = [None, None]

    def gn_stage(b):
        # two half-loads on separate DMA queues; per-half bn_stats (DVE)
        # computes mean+M2 in one pass and starts on the first half
        xt = xpool.tile([P, CT, N], mmdt, tag="xt")
        xsrc = io["x"][b].rearrange("(t p) n -> p t n", p=P)
        nc.sync.dma_start(out=xt[:, 0 : CT // 2], in_=xsrc[:, 0 : CT // 2])
        nc.scalar.dma_start(out=xt[:, CT // 2 :], in_=xsrc[:, CT // 2 :])

        # stats8 = [mean_ct | E[x^2]_ct] via bn_stats (single-pass mean+M2
        # on DVE). Batch 1's bnst tile shares a bufs=1 slot with a tiny
        # "gate" tile written at the end of batch 0's stats->xn chain, so
        # the scheduler cannot slip batch 1's 600ns bn_stats ops into DVE
        # idle moments inside batch 0's critical chain.
        stats8 = small.tile([P, 2 * CT], F32, tag="stats8")
        if b == 0:
            bnst = small.tile([P, 2 * CT, 6], F32, tag="bnst")
        else:
            bnst = gatepool.tile([P, 2 * CT, 6], F32, tag="g", name="bnst1")
        xv = xt.rearrange("p c (h f) -> p (c h) f", f=512)
        for h in range(2 * CT):  # hw cap: bn_stats free size <= 512
            nc.vector.bn_stats(out=bnst[:, h], in_=xv[:, h])
        mv = small.tile([P, CT, 2], F32, tag="mv")
        for ct in range(CT):
            nc.vector.bn_aggr(out=mv[:, ct], in_=bnst[:, 2 * ct : 2 * ct + 2])
        nc.vector.tensor_copy(out=stats8[:, 0:CT], in_=mv[:, :, 0])
        musq0 = small.tile([P, CT], F32, tag="musq0")
        nc.vector.tensor_mul(out=musq0, in0=mv[:, :, 0], in1=mv[:, :, 0])
        nc.vector.tensor_tensor(
            out=stats8[:, CT : 2 * CT],
            in0=mv[:, :, 1],
            in1=musq0,
            op=mybir.AluOpType.add,
        )

        # group-average broadcast back to channels: one tiny matmul with
        # gmat = blockdiag(1/GS) -> [mu_g | E_g[x^2]] per channel
        pgf = psum.tile([P, NCH, 512], F32, tag="mm", name="pgf")
        pg = pgf[:, 0, : 2 * CT]
        nc.tensor.matmul(pg, lhsT=gmat, rhs=stats8, start=True, stop=True)
        if b == 0:
            # PE p-state bridge: the tensor engine clock drops back to
            # 0.65GHz after an idle gap and takes 3us of continuous work to
            # re-reach 2.4GHz. Keep it warm through the stats->xn window
            # with throwaway matmuls into pgf's unused bank so the first
            # real qkv matmuls run at speed.
            for i in range(6):
                nc.tensor.matmul(
                    pgf[:, 1, 0:128],
                    lhsT=gmat,
                    rhs=cpak[:, 0:128],
                    start=True,
                    stop=True,
                )

        mu = pg[:, 0:CT]
        ex2 = pg[:, CT : 2 * CT]
        # mu^2 on ScalarE: a DVE tensor_mul(mu, mu) would read PSUM twice,
        # which the hardware (walrus NCC_IBVF027) forbids
        musq = small.tile([P, CT], F32, tag="musq")
        nc.scalar.activation(out=musq, in_=mu, func=AF.Square)
        var = small.tile([P, CT], F32, tag="var")
        nc.vector.tensor_sub(out=var, in0=ex2, in1=musq)
        # rstd = exp(-0.5*ln(var+eps)): stays on the exp/square/identity
        # activation table (Sqrt would force a table reload each pass)
        lnv = small.tile([P, CT], F32, tag="lnv")
        nc.scalar.activation(out=lnv, in_=var, func=AF.Ln, bias=epsc)
        rstd = small.tile([P, CT], F32, tag="rstd")
        nc.scalar.activation(out=rstd, in_=lnv, func=AF.Exp, scale=-0.5)
        if gn_identity:
            a44 = rstd
            mus = small.tile([P, CT], F32, tag="mus")
            nc.vector.tensor_mul(out=mus, in0=mu, in1=rstd)
            d44 = small.tile([P, CT], F32, tag="d44")
            nc.vector.tensor_scalar(
                out=d44, in0=mus, scalar1=-1.0, scalar2=None,
                op0=mybir.AluOpType.mult,
            )
        else:
            a44 = small.tile([P, CT], F32, tag="a44")
            nc.vector.tensor_mul(out=a44, in0=rstd, in1=gnw)
            tmp44 = small.tile([P, CT], F32, tag="tmp44")
            nc.vector.tensor_mul(out=tmp44, in0=mu, in1=a44)
            d44 = small.tile([P, CT], F32, tag="d44")
            nc.vector.tensor_sub(out=d44, in0=gnb, in1=tmp44)
        if b == 0:
            # end-of-chain marker; releases the gatepool slot that batch
            # 1's bnst tile waits on (see stats8 comment above)
            gate = gatepool.tile([P, 2 * CT, 6], F32, tag="g", name="gate0")
            nc.vector.tensor_copy(out=gate[:, 0, 0:1], in_=d44[:, 0:1])

        # xn slices split across ScalarE and DVE so the first qkv matmul
        # (gated on xn[:, 0]) unblocks in ~1us instead of 4 serial passes
        xn = xnpool.tile([P, CT, N], mmdt, tag="xn")
        for ct in range(CT):
            if ct % 2 == 0:
                nc.scalar.activation(
                    out=xn[:, ct],
                    in_=xt[:, ct],
                    func=AF.Identity,
                    bias=d44[:, ct : ct + 1],
                    scale=a44[:, ct : ct + 1],
                )
            else:
                nc.vector.tensor_scalar(
                    out=xn[:, ct],
                    in0=xt[:, ct],
                    scalar1=a44[:, ct : ct + 1],
                    scalar2=d44[:, ct : ct + 1],
                    op0=mybir.AluOpType.mult,
                    op1=mybir.AluOpType.add,
                )
        xn_tiles[b] = xn

    # --- phase B: attention, software-pipelined across the two batches ---
    # Stage order interleaves the batches so PE keeps matmul work queued
    # while a batch sits in softmax (ScalarE) or transpose (DMA xbar).
    qd, kd, vd, ptd = {}, {}, {}, {}

    def qkv_stage(b):
        xn = xn_tiles[b]
        q = qkpool.tile([P, CT, N], mmdt, tag="q", name="q")
        k = qkpool.tile([P, CT, N], mmdt, tag="k", name="k")
        qd[b], kd[b] = q, k
        if b == 0:
            # First PE stage after GroupNorm: open all four q accumulation
            # groups on the kc=0 pass so PE has 8 matmuls of work the
            # moment xn[:, 0] lands, instead of stalling per group on the
            # remaining xn slices.
            pqs = []
            for ct in range(CT):
                pq = psum.tile([P, NCH, 512], F32, tag="mm", name="pq")
                pqs.append(pq)
                for h in range(NCH):
                    nc.tensor.matmul(
                        pq[:, h],
                        lhsT=wqkv[:, 0, ct * P : (ct + 1) * P],
                        rhs=xn[:, 0, h * 512 : (h + 1) * 512],
                        start=True,
                        stop=False,
                    )
            q_groups = [(pqs[ct], range(1, CT)) for ct in range(CT)]
        else:
            q_groups = None
        for ct in range(CT):
            if q_groups is not None:
                pq, kcs = q_groups[ct]
            else:
                pq, kcs = (
                    psum.tile([P, NCH, 512], F32, tag="mm", name="pq"),
                    range(CT),
                )
            for h in range(NCH):
                for kc in kcs:
                    nc.tensor.matmul(
                        pq[:, h],
                        lhsT=wqkv[:, kc, ct * P : (ct + 1) * P],
                        rhs=xn[:, kc, h * 512 : (h + 1) * 512],
                        start=(kc == 0),
                        stop=(kc == CT - 1),
                    )
            if zero_bias:
                nc.vector.tensor_scalar(
                    out=q[:, ct],
                    in0=pq.rearrange("p h m -> p (h m)"),
                    scalar1=QK_SCALE,
                    scalar2=None,
                    op0=mybir.AluOpType.mult,
                )
            else:
                nc.vector.tensor_scalar(
                    out=q[:, ct],
                    in0=pq.rearrange("p h m -> p (h m)"),
                    scalar1=qb[:, ct : ct + 1],
                    scalar2=QK_SCALE,
                    op0=mybir.AluOpType.add,
                    op1=mybir.AluOpType.mult,
                )
            pk = psum.tile([P, NCH, 512], F32, tag="mm", name="pk")
            for h in range(NCH):
                for kc in range(CT):
                    nc.tensor.matmul(
                        pk[:, h],
                        lhsT=wqkv[:, kc, C + ct * P : C + (ct + 1) * P],
                        rhs=xn[:, kc, h * 512 : (h + 1) * 512],
                        start=(kc == 0),
                        stop=(kc == CT - 1),
                    )
            if zero_bias:
                nc.vector.tensor_copy(
                    out=k[:, ct], in_=pk.rearrange("p h m -> p (h m)")
                )
            else:
                nc.vector.tensor_scalar(
                    out=k[:, ct],
                    in0=pk.rearrange("p h m -> p (h m)"),
                    scalar1=kb[:, ct : ct + 1],
                    op0=mybir.AluOpType.add,
                )
        vT = vpool.tile([P, NT, C], BF16, tag="vT", name="vT")
        vd[b] = vT
        for mp in range(NT // 2):
            pv = psum.tile([P, NCH, 512], F32, tag="mm", name="pv")
            for i in range(2):
                mt = 2 * mp + i
                for kc in range(CT):
                    nc.tensor.matmul(
                        pv[:, i],
                        lhsT=xn[:, kc, mt * P : (mt + 1) * P],
                        rhs=wqkv[:, kc, 2 * C : 3 * C],
                        start=(kc == 0),
                        stop=(kc == CT - 1),
                    )
            if zero_bias:
                nc.vector.tensor_copy(
                    out=vT[:, 2 * mp : 2 * mp + 2], in_=pv
                )
            else:
                nc.vector.tensor_tensor(
                    out=vT[:, 2 * mp : 2 * mp + 2],
                    in0=pv,
                    in1=vb[:, None, :].to_broadcast([P, 2, C]),
                    op=mybir.AluOpType.add,
                )

    def smax_stage(b):
        # Per-strip softmax: exp with a row-sum accumulator, a tiny
        # reciprocal + one 1024-wide renorm multiply, then the strip's
        # DMA-xbar transpose starts immediately — no batch-wide barrier
        # between exp and transpose.
        q, k = qd[b], kd[b]
        pmat = ppool.tile([P, NT, N], BF16, tag="P", name="pmat")
        pmatT = ptpool.tile([P, NT, N], BF16, tag="PT", name="pmatT")
        ptd[b] = pmatT
        lsum8 = small.tile([P, NT], F32, tag="lsum8")
        rsum8 = small.tile([P, NT], F32, tag="rsum8")
        for nt in range(NT):
            ps = psum.tile([P, NCH, 512], F32, tag="mm", name="ps")
            for h in range(NCH):
                for kc in range(CT):
                    nc.tensor.matmul(
                        ps[:, h],
                        lhsT=q[:, kc, nt * P : (nt + 1) * P],
                        rhs=k[:, kc, h * 512 : (h + 1) * 512],
                        start=(kc == 0),
                        stop=(kc == CT - 1),
                    )
            nc.scalar.activation(
                out=pmat[:, nt],
                in_=ps.rearrange("p h m -> p (h m)"),
                func=AF.Exp,
                bias=zeroc,
                scale=1.0,
                accum_out=lsum8[:, nt : nt + 1],
            )
            nc.vector.reciprocal(
                out=rsum8[:, nt : nt + 1], in_=lsum8[:, nt : nt + 1]
            )
            nc.vector.tensor_tensor(
                out=pmat[:, nt],
                in0=pmat[:, nt],
                in1=rsum8[:, nt : nt + 1].to_broadcast([P, N]),
                op=mybir.AluOpType.mult,
            )
            # PT[pm, mt, nt*128+nn] = P[nn, nt, mt*128+pm]
            nc.sync.dma_start_transpose(
                out=pmatT[:, :, nt * P : (nt + 1) * P],
                in_=pmat[:, nt],
            )

    def out_stage(b):
        vT, pmatT = vd[b], ptd[b]
        o = opool.tile([P, CT, N], mmdt, tag="o", name="o")
        for ct in range(CT):
            po = psum.tile([P, NCH, 512], F32, tag="mm", name="po")
            for h in range(NCH):
                for mt in range(NT):
                    nc.tensor.matmul(
                        po[:, h],
                        lhsT=vT[:, mt, ct * P : (ct + 1) * P],
                        rhs=pmatT[:, mt, h * 512 : (h + 1) * 512],
                        start=(mt == 0),
                        stop=(mt == NT - 1),
                    )
            nc.scalar.activation(
                out=o[:, ct],
                in_=po.rearrange("p h m -> p (h m)"),
                func=AF.Copy,
            )
        yt = ypool.tile([P, CT, N], F32, tag="yt", name="yt")
        ydram = io["y"][b].rearrange("(t p) n -> p t n", p=P)
        for ct in range(CT):
            pp = psum.tile([P, NCH, 512], F32, tag="mm", name="pp")
            for h in range(NCH):
                for kc in range(CT):
                    nc.tensor.matmul(
                        pp[:, h],
                        lhsT=wproj[:, kc, ct * P : (ct + 1) * P],
                        rhs=o[:, kc, h * 512 : (h + 1) * 512],
                        start=(kc == 0),
                        stop=(kc == CT - 1),
                    )
            if zero_bias:
                nc.vector.tensor_copy(
                    out=yt[:, ct], in_=pp.rearrange("p h m -> p (h m)")
                )
            else:
                nc.vector.tensor_scalar(
                    out=yt[:, ct],
                    in0=pp.rearrange("p h m -> p (h m)"),
                    scalar1=pb[:, ct : ct + 1],
                    op0=mybir.AluOpType.add,
                )
            yeng = nc.sync if ct % 2 == 0 else nc.scalar
            yeng.dma_start(out=ydram[:, ct], in_=yt[:, ct])

    gn_stage(0)
    load_weights()
    qkv_stage(0)
    gn_stage(1)
    smax_stage(0)
    qkv_stage(1)
    smax_stage(1)
    out_stage(0)
    out_stage(1)

    ctx.close()


def build(legalize=True, reps=1, loop_reps=None, unroll=1, drop_ldw=False,
          zero_bias=True, gn_identity=True):
    """loop_reps=N wraps ONE static copy of the body in a tc.For_i(0, N)
    hardware loop: the device executes the body N times while the program
    (and thus client/tunnel dispatch cost, which scales with static
    instruction count) stays constant-size. test.py uses this to measure
    true marginal device-execution time per body."""
    _apply_tile_patch()
    nc = bass.Bass(
        "TRN2", target_bir_lowering=False, debug=False, num_devices=N_CORES
    )
    kw = dict(zero_bias=zero_bias, gn_identity=gn_identity)
    with tile.TileContext(nc) as tc:
        io = _declare_io(nc)
        if loop_reps is not None:
            with tc.For_i(0, loop_reps):
                for u in range(unroll):
                    _emit(tc, io, rt=f"_u{u}" if u else "", **kw)
        else:
            for r in range(reps):
                _emit(tc, io, rt=f"_{r}" if r else "", **kw)
    if legalize:
        _legalize_waits(nc, drop_ldw=drop_ldw)
    return nc


# ---------------------------------------------------------------------------
# Host-side entry point
# ---------------------------------------------------------------------------
def _host_inputs(x, gn_weight, gn_bias, qkv_weight, qkv_bias, proj_weight,
                 proj_bias):
    import ml_dtypes

    mmnp = ml_dtypes.bfloat16 if MM_MODE == "bf16" else np.float32
    x = np.asarray(x, dtype=np.float32).reshape(B, C, N).astype(mmnp)
    qkv_weight = np.asarray(qkv_weight, dtype=np.float32)
    proj_weight = np.asarray(proj_weight, dtype=np.float32)

    def p44(v):
        return np.ascontiguousarray(
            np.asarray(v, dtype=np.float32).reshape(CT, P).T
        )

    wqkvT = np.ascontiguousarray(qkv_weight.T.astype(mmnp))
    wprojT = np.ascontiguousarray(
        (proj_weight + np.eye(C, dtype=np.float32)).T.astype(mmnp)
    )
    # bn_stats feeds per-channel means, so group averaging is just 1/GS
    gmat = np.zeros((P, P), dtype=np.float32)
    for g in range(P // GS):
        gmat[g * GS : (g + 1) * GS, g * GS : (g + 1) * GS] = 1.0 / GS

    qkv_bias = np.asarray(qkv_bias, dtype=np.float32)
    cpak = np.concatenate(
        [
            p44(gn_weight),
            p44(gn_bias),
            p44(qkv_bias[0:C]),
            p44(qkv_bias[C : 2 * C]),
            p44(proj_bias),
            np.full((P, 1), EPS, np.float32),
            np.zeros((P, 1), np.float32),
            gmat,
            np.broadcast_to(qkv_bias[2 * C : 3 * C], (P, C)),
        ],
        axis=1,
    )
    shared = {
        "wqkvT": wqkvT,
        "wprojT": wprojT,
        "cpak": np.ascontiguousarray(cpak),
    }
    in_maps = []
    for i in range(N_CORES):
        m = dict(shared)
        m["x"] = np.ascontiguousarray(x[i * BL : (i + 1) * BL])
        in_maps.append(m)
    return in_maps


def _fast_flags(gn_weight, gn_bias, qkv_bias, proj_bias):
    """Build-time specialization on the supplied parameter values; the
    general path handles anything else."""
    gn_identity = bool(
        np.all(np.asarray(gn_weight) == 1.0) and np.all(np.asarray(gn_bias) == 0.0)
    )
    zero_bias = bool(
        np.all(np.asarray(qkv_bias) == 0.0) and np.all(np.asarray(proj_bias) == 0.0)
    )
    return zero_bias, gn_identity


_NC_KEY = None
_NC = None
_RUNNER = None


def _make_runner(nc):
    """Cached PJRT executor: the jitted shard_map is built once; shared
    weight operands are broadcast (uploaded once, not 8x); the zeroed
    output-donation buffers live on device and are reused every call."""
    import jax
    import concourse.mybir as mb
    from concourse import bass2jax
    from concourse.bass2jax import (
        _bass_exec_p,
        install_neuronx_cc_hook,
        partition_id_tensor,
    )
    from jax.experimental.shard_map import shard_map
    from jax.sharding import Mesh, NamedSharding, PartitionSpec

    install_neuronx_cc_hook()

    pid_name = (
        nc.partition_id_tensor.name if nc.partition_id_tensor else None
    )
    in_names, out_names, out_avals, zero_outs = [], [], [], []
    for alloc in nc.m.functions[0].allocations:
        if not isinstance(alloc, mb.MemoryLocationSet):
            continue
        name = alloc.memorylocations[0].name
        if alloc.kind == "ExternalInput":
            if name == pid_name:
                continue
            in_names.append(name)
        elif alloc.kind == "ExternalOutput":
            out_names.append(name)
            shape = tuple(alloc.tensor_shape)
            dtype = mb.dt.np(alloc.dtype)
            out_avals.append(jax.core.ShapedArray(shape, dtype))
            zero_outs.append(np.zeros(shape, dtype))
    n_params = len(in_names)
    all_in_names = in_names + out_names
    if pid_name is not None:
        all_in_names = all_in_names + [pid_name]

    def _body(*args):
        operands = list(args)
        if pid_name is not None:
            operands.append(partition_id_tensor())
        outs = _bass_exec_p.bind(
            *operands,
            out_avals=tuple(out_avals),
            in_names=tuple(all_in_names),
            out_names=tuple(out_names),
            lowering_input_output_aliases=(),
            sim_require_finite=True,
            sim_require_nnan=True,
            nc=nc,
        )
        return tuple(outs)

    devices = jax.devices()[:N_CORES]
    mesh = Mesh(np.asarray(devices), ("core",))
    sharded_names = {"x"}
    in_specs = tuple(
        PartitionSpec("core") if nm in sharded_names else PartitionSpec()
        for nm in in_names
    ) + (PartitionSpec("core"),) * len(out_names)
    out_specs = (PartitionSpec("core"),) * len(out_names)
    fn = jax.jit(
        shard_map(
            _body, mesh=mesh, in_specs=in_specs, out_specs=out_specs,
            check_rep=False,
        ),
        keep_unused=True,
    )
    zeros_dev = [
        jax.device_put(
            np.zeros((N_CORES * z.shape[0], *z.shape[1:]), z.dtype),
            NamedSharding(mesh, PartitionSpec("core")),
        )
        for z in zero_outs
    ]

    def run(in_maps):
        ins = []
        for nm in in_names:
            if nm in sharded_names:
                ins.append(
                    np.concatenate([m[nm] for m in in_maps], axis=0)
                )
            else:
                ins.append(in_maps[0][nm])
        outs = fn(*ins, *zeros_dev)
        return [np.asarray(o) for o in outs], out_names

    return run


def kernel(x, gn_weight, gn_bias, qkv_weight, qkv_bias, proj_weight,
           proj_bias, _trace=False, _results=None):
    global _NC, _RUNNER, _NC_KEY
    zero_bias, gn_identity = _fast_flags(
        gn_weight, gn_bias, qkv_bias, proj_bias
    )
    key = (zero_bias, gn_identity)
    if _NC is None or _NC_KEY != key:
        _NC = build(zero_bias=zero_bias, gn_identity=gn_identity)
        _RUNNER = _make_runner(_NC)
        _NC_KEY = key
    in_maps = _host_inputs(
        x, gn_weight, gn_bias, qkv_weight, qkv_bias, proj_weight, proj_bias
    )
    outs, out_names = _RUNNER(in_maps)
    y = outs[out_names.index("y")]  # [N_CORES*BL, C, N]
    return y.reshape(B, C, H, W).astype(np.float32)

